# revision 1
# baseline (speedup 1.0000x reference)
"""GCN encoder (2x GCNConv + ReLU + global mean pool) as a Bass SPMD kernel
for 8 trn2 NeuronCores.

Formulation (per layer, A includes self loops, D = degree over dest):
    out = D^-1/2 A D^-1/2 (x W + b)   with b == 0 enforced
        = dinv * (AGG @ W)            AGG[n] = sum_{e: row=n} T[col_e],
                                      T = dinv * x   (layer input scaled)
Layer 1: T2 = dinv * relu(out1) = dinv^2 * relu(AGG1 @ W1)
Layer 2: out2 = dinv * (AGG2 @ W2); pooled = segsum(out2, batch) / cnt

Distribution: nodes block-sharded over 8 cores. Each core aggregates its
own destination rows. The scaled-feature table T (bf16, all nodes) is
AllGathered between layers; the layer-2 table uses a quarter-major layout
so each quarter's AllGather fires mid-sweep-1 and hides under compute.
Edge gathers use the GPSIMD dma_gather custom instruction (int16 idx, so
the table is split into <=32K-row windows); self-loop edges skip the
gather entirely via an identity-matrix matmul on local (sequential) rows.

SPMD constraint: one program runs on all 8 cores, so the loop structure is
uniform. Group capacities are max'ed across cores; surplus idx slots point
at row 0 of their table window with row_local=-1 (zeroed by the selection
matrix). A per-core greedy node->slot permutation flattens group sizes to
shrink that capacity padding.
"""
import math
import numpy as np
import ml_dtypes

import concourse.bass as bass
import concourse.mybir as mybir
import concourse.tile as tile
from concourse import bacc

P = 128
NCORE = 8
bf16 = mybir.dt.bfloat16
f32 = mybir.dt.float32
i16 = mybir.dt.int16


class Cfg:
    def __init__(self, n_nodes, n_graphs, sb_blocks=4, max_tab_rows=25088):
        assert n_nodes % NCORE == 0
        self.N = n_nodes
        self.G = n_graphs
        self.n_sh = n_nodes // NCORE                     # owned nodes per core
        self.nblk = math.ceil(self.n_sh / P)             # blocks per core
        self.n_shp = self.nblk * P                       # padded shard rows
        self.nt_full = NCORE * self.n_shp                # padded table rows
        # int16 gather windows: split table into <=32768-row windows
        self.ntab = 1
        while (self.nt_full / self.ntab > min(max_tab_rows, 32000)
               or self.n_shp % self.ntab or self.nt_full % self.ntab):
            self.ntab += 1
            assert self.ntab <= 64
        self.tab_rows = self.nt_full // self.ntab
        self.qrows = self.n_shp // self.ntab             # shard rows per quarter
        self.sb_blocks = sb_blocks
        self.nsb = math.ceil(self.nblk / sb_blocks)
        # t2 AllGather pipelining: split the shard into nag contiguous
        # chunks whose windows nest inside the int16 gather windows
        self.nag = 2 if (self.nblk % 2 == 0 and self.ntab % 2 == 0
                         and self.n_sh % 2 == 0) else 1
        self.hrows = self.n_shp // self.nag
        assert self.G <= 2 * P


def _layer_structure(cfg, core_of, blk, rl, tab, tab_off, col):
    """Uniform call/piece structure + per-core idx/rl tiles for one layer.

    A "piece" is (block, gather-column, p0, p1, rl-column, is_last): one
    full-K matmul of gather column `coli` into block b's psum, with a
    dedicated rl column that is -1 outside [p0,p1) so the selection matrix
    zeroes other blocks' slots sharing the column.
    """
    order = np.lexsort((col, tab, blk, core_of))
    core_s, blk_s, tab_s, rl_s, off_s = (
        core_of[order], blk[order], tab[order], rl[order], tab_off[order])

    sizes = np.zeros((NCORE, cfg.nblk, cfg.ntab), dtype=np.int64)
    np.add.at(sizes, (core_s, blk_s, tab_s), 1)
    caps = sizes.max(axis=0)                      # [nblk, ntab]

    grp_start = np.zeros((NCORE, cfg.nblk, cfg.ntab), dtype=np.int64)
    grp_start.reshape(-1)[1:] = np.cumsum(sizes.reshape(-1))[:-1]

    calls = []
    icol = 0   # idx tile column cursor (16 idxs per column)
    pcol = 0   # rl tile column cursor (one per piece)
    for sb in range(cfg.nsb):
        blocks = range(sb * cfg.sb_blocks, min((sb + 1) * cfg.sb_blocks, cfg.nblk))
        for t in range(cfg.ntab):
            cap = int(sum(caps[b, t] for b in blocks))
            if cap == 0:
                continue
            cap16 = ((cap + 15) // 16) * 16       # idx tile is 16-wrapped
            ncol = (cap16 + P - 1) // P
            pieces = []
            off = 0
            groups = []
            for b in blocks:
                c = int(caps[b, t])
                if c == 0:
                    continue
                groups.append((b, off, c))
                pos = off
                while pos < off + c:
                    coli = pos // P
                    p0 = pos % P
                    take = min(P - p0, off + c - pos)
                    pieces.append([b, coli, p0, p0 + take, pcol, False])
                    pcol += 1
                    pos += take
                off += c
            assert off == cap
            calls.append(dict(sb=sb, t=t, cap=cap16, icol=icol, ncol=ncol,
                              pieces=pieces, groups=groups))
            icol += cap16 // 16
    icols, pcols = icol, pcol

    # mark last piece per block across the layer order (psum stop flag)
    last_piece = {}
    for call in calls:
        for pc in call["pieces"]:
            last_piece[pc[0]] = pc
    for pc in last_piece.values():
        pc[5] = True

    idx_all = np.zeros((NCORE, 16, icols), dtype=np.int16)
    rl_all = np.full((NCORE, P, pcols), -1.0, dtype=np.float32)
    for call in calls:
        t = call["t"]
        grp_of_block = {b: (so, cp) for b, so, cp in call["groups"]}
        for pc in call["pieces"]:
            b, coli, p0, p1, pci, _ = pc
            slot_off, gcap = grp_of_block[b]
            for c in range(NCORE):
                n = int(sizes[c, b, t])
                s0 = grp_start[c, b, t]
                g_lo = coli * P + p0 - slot_off
                g_hi = coli * P + p1 - slot_off
                lo, hi = max(g_lo, 0), min(g_hi, n)
                if lo < hi:
                    rl_all[c][p0 + (lo - g_lo):p0 + (hi - g_lo), pci] = \
                        rl_s[s0 + lo:s0 + hi]
        for b, slot_off, gcap in call["groups"]:
            base = call["icol"] * 16 + slot_off
            for c in range(NCORE):
                n = int(sizes[c, b, t])
                s0 = grp_start[c, b, t]
                if n:
                    pos = base + np.arange(n)
                    idx_all[c][pos % 16, pos // 16] = off_s[s0:s0 + n].astype(np.int16)
                # pad slots stay 0 in idx (row 0 of window)

    return dict(
        calls=calls, icols=icols, ccols=pcols,
        idx_tiles=[np.tile(idx_all[c], (8, 1)) for c in range(NCORE)],
        rl_tiles=[rl_all[c].astype(ml_dtypes.bfloat16) for c in range(NCORE)])


def host_prep(cfg, edge_index, batch):
    N, G = cfg.N, cfg.G
    row = np.asarray(edge_index[0], dtype=np.int64)
    col = np.asarray(edge_index[1], dtype=np.int64)
    # degree over col including self loops
    deg = np.bincount(col, minlength=N).astype(np.float32) + 1.0

    core_of = row // cfg.n_sh
    src_core = col // cfg.n_sh

    # --- per-core greedy node->slot permutation: flatten per-(block, table)
    # group sizes so the cross-core capacity max is tight. Balancing uses
    # the unpermuted layer-1 table id (close enough for both layers).
    # Both layers' table ids of an edge must be permutation-invariant so
    # the greedy can balance them exactly: L1 window = owner group (shard-
    # aligned windows), L2 window = AG-chunk(col) * windows-per-chunk +
    # owner subgroup, with the permutation constrained to keep each node in
    # its original AG chunk ("half").
    nag = cfg.nag
    pool_sz = cfg.n_sh // nag
    half_of_node = np.minimum(np.arange(cfg.n_sh) // pool_sz, nag - 1)
    t1_of = src_core // max(NCORE // cfg.ntab, 1)
    wpc = cfg.ntab // nag                     # windows per AG chunk
    spw = NCORE // wpc                        # source cores per window
    t2_of = half_of_node[col % cfg.n_sh] * wpc + src_core // spw

    d8 = np.zeros((N, 2 * cfg.ntab), dtype=np.int32)
    np.add.at(d8, (row, t1_of), 1)
    np.add.at(d8, (row, cfg.ntab + t2_of), 1)

    perm = np.full((NCORE, cfg.n_shp), -1, dtype=np.int64)   # slot -> local node
    inv = np.zeros((NCORE, cfg.n_sh), dtype=np.int64)        # local node -> slot
    blk_per_half = cfg.nblk // nag
    for c in range(NCORE):
        dall = d8[c * cfg.n_sh:(c + 1) * cfg.n_sh].astype(np.float64)
        for h in range(nag):
            nodes = np.where(half_of_node == h)[0]
            d = dall[nodes]
            order_n = np.argsort(-d.sum(1), kind="stable")
            target = d.sum(0) / blk_per_half + 1e-9
            sums = np.zeros((blk_per_half, 2 * cfg.ntab))
            fill = np.zeros(blk_per_half, dtype=np.int64)
            b0 = h * blk_per_half
            for i in order_n:
                n = nodes[i]
                score = ((sums + d[i]) / target).max(axis=1)
                score[fill >= P] = np.inf
                b = int(np.argmin(score))
                sums[b] += d[i]
                perm[c, (b0 + b) * P + fill[b]] = n
                inv[c, n] = (b0 + b) * P + fill[b]
                fill[b] += 1

    r_loc = inv[core_of, row % cfg.n_sh]
    blk = r_loc // P
    rl = r_loc % P
    src_slot = inv[src_core, col % cfg.n_sh]

    # layer 1 table: shard-concat [core, slot]
    src_row1 = src_core * cfg.n_shp + src_slot
    l1 = _layer_structure(cfg, core_of, blk, rl,
                          src_row1 // cfg.tab_rows, src_row1 % cfg.tab_rows, col)
    # layer 2 table: chunk-major [chunk, core, slot % hrows]; windows nest
    # inside chunks so window t == chunk*wpc + subrange
    row2 = ((src_slot // cfg.hrows) * (NCORE * cfg.hrows)
            + src_core * cfg.hrows + (src_slot % cfg.hrows))
    l2 = _layer_structure(cfg, core_of, blk, rl,
                          row2 // cfg.tab_rows, row2 % cfg.tab_rows, col)

    batch = np.asarray(batch, dtype=np.int64)
    deg_t, bt = [], []
    for c in range(NCORE):
        pc = perm[c]
        valid = pc >= 0
        d = np.ones(cfg.n_shp, dtype=np.float32)
        d[valid] = deg[c * cfg.n_sh + pc[valid]]
        deg_t.append(np.ascontiguousarray(d.reshape(cfg.nblk, P).T))
        b = np.full(cfg.n_shp, -1.0, dtype=np.float32)
        b[valid] = batch[c * cfg.n_sh + pc[valid]]
        bt.append(np.ascontiguousarray(
            b.reshape(cfg.nblk, P).T).astype(ml_dtypes.bfloat16))

    cnts = np.bincount(batch, minlength=G).astype(np.float32)
    inv_pad = np.zeros(2 * P, dtype=np.float32)
    inv_pad[:G] = 1.0 / np.maximum(cnts, 1.0)
    inv_tile = np.ascontiguousarray(inv_pad.reshape(2, P).T)  # [128, 2]

    return dict(l1=l1, l2=l2, deg_t=deg_t, batch_t=bt, inv_tile=inv_tile,
                perm=perm)


def build_program(cfg, prep):
    nc = bacc.Bacc("TRN2", target_bir_lowering=False, num_devices=NCORE,
                   num_swdge_queues=4)
    nblk, nsb = cfg.nblk, cfg.nsb
    l1, l2 = prep["l1"], prep["l2"]

    x_in = nc.declare_dram_parameter("x_local", [cfg.n_shp, P], f32, isOutput=False)
    w1_in = nc.declare_dram_parameter("w1", [P, P], f32, isOutput=False)
    w2_in = nc.declare_dram_parameter("w2", [P, P], f32, isOutput=False)
    deg_in = nc.declare_dram_parameter("deg_t", [P, nblk], f32, isOutput=False)
    iota_lo_in = nc.declare_dram_parameter("iota_lo", [P, P], bf16, isOutput=False)
    iota_hi_in = nc.declare_dram_parameter("iota_hi", [P, P], bf16, isOutput=False)
    ident_in = nc.declare_dram_parameter("ident", [P, P], bf16, isOutput=False)
    idx1_in = nc.declare_dram_parameter("idx1", [P, l1["icols"]], i16, isOutput=False)
    rl1_in = nc.declare_dram_parameter("rl1", [P, l1["ccols"]], bf16, isOutput=False)
    idx2_in = nc.declare_dram_parameter("idx2", [P, l2["icols"]], i16, isOutput=False)
    rl2_in = nc.declare_dram_parameter("rl2", [P, l2["ccols"]], bf16, isOutput=False)
    batch_in = nc.declare_dram_parameter("batch_t", [P, nblk], bf16, isOutput=False)
    invc_in = nc.declare_dram_parameter("inv_cnt", [P, 2], f32, isOutput=False)
    out_ext = nc.declare_dram_parameter("out", [2 * P, P], f32, isOutput=True)

    t1_shard = nc.dram_tensor("t1_shard", [cfg.n_shp, P], bf16)
    t1_full = nc.dram_tensor("t1_full", [cfg.nt_full, P], bf16, addr_space="Shared")
    t2_shard = nc.dram_tensor("t2_shard", [cfg.n_shp, P], bf16)
    t2_full = nc.dram_tensor("t2_full", [cfg.nt_full, P], bf16, addr_space="Shared")
    pool_part = nc.dram_tensor("pool_part", [2 * P, P], f32)
    pool_full = nc.dram_tensor("pool_full", [2 * P, P], f32, addr_space="Shared")

    with tile.TileContext(nc) as tc:
        with tc.tile_pool(name="const", bufs=1) as cpool, \
             tc.tile_pool(name="xio", bufs=3) as xpool, \
             tc.tile_pool(name="gath", bufs=8) as gpool, \
             tc.tile_pool(name="sel", bufs=6) as spool, \
             tc.tile_pool(name="blk", bufs=4) as bpool, \
             tc.tile_pool(name="agg", bufs=4, space="PSUM") as apool, \
             tc.tile_pool(name="hp", bufs=2, space="PSUM") as hpool, \
             tc.tile_pool(name="pool", bufs=1, space="PSUM") as ppool:

            # ---- constants ----
            iota_lo = cpool.tile([P, P], bf16)
            nc.sync.dma_start(out=iota_lo[:], in_=iota_lo_in[:])
            iota_hi = cpool.tile([P, P], bf16)
            nc.sync.dma_start(out=iota_hi[:], in_=iota_hi_in[:])
            ident = cpool.tile([P, P], bf16)
            nc.sync.dma_start(out=ident[:], in_=ident_in[:])
            idx1_sb = cpool.tile([P, l1["icols"]], i16)
            nc.sync.dma_start(out=idx1_sb[:], in_=idx1_in[:])
            rl1_sb = cpool.tile([P, l1["ccols"]], bf16)
            nc.sync.dma_start(out=rl1_sb[:], in_=rl1_in[:])
            idx2_sb = cpool.tile([P, l2["icols"]], i16)
            nc.sync.dma_start(out=idx2_sb[:], in_=idx2_in[:])
            rl2_sb = cpool.tile([P, l2["ccols"]], bf16)
            nc.sync.dma_start(out=rl2_sb[:], in_=rl2_in[:])
            batch_sb = cpool.tile([P, nblk], bf16)
            nc.sync.dma_start(out=batch_sb[:], in_=batch_in[:])
            invc_sb = cpool.tile([P, 2], f32)
            nc.sync.dma_start(out=invc_sb[:], in_=invc_in[:])

            w1f = cpool.tile([P, P], f32)
            nc.sync.dma_start(out=w1f[:], in_=w1_in[:])
            w1_sb = cpool.tile([P, P], bf16)
            nc.vector.tensor_copy(out=w1_sb[:], in_=w1f[:])
            w2f = cpool.tile([P, P], f32)
            nc.sync.dma_start(out=w2f[:], in_=w2_in[:])
            w2_sb = cpool.tile([P, P], bf16)
            nc.vector.tensor_copy(out=w2_sb[:], in_=w2f[:])

            degf = cpool.tile([P, nblk], f32)
            nc.sync.dma_start(out=degf[:], in_=deg_in[:])
            sq = cpool.tile([P, nblk], f32)
            nc.scalar.sqrt(out=sq[:], in_=degf[:])
            dinv = cpool.tile([P, nblk], f32)
            nc.vector.reciprocal(out=dinv[:], in_=sq[:])
            dinv2 = cpool.tile([P, nblk], f32)
            nc.vector.tensor_mul(out=dinv2[:], in0=dinv[:], in1=dinv[:])

            # zero-init gather ring buffers (stale tails must be finite)
            max_ncol = max(c["ncol"] for c in l1["calls"] + l2["calls"])
            for _ in range(8):
                gz = gpool.tile([P, max_ncol, P], bf16, tag="g")
                nc.vector.memset(gz[:], 0.0)

            # ---- T1 = dinv * x (local shard), in multi-block slabs ----
            slab = 1
            for s in (14, 7, 2, 1):
                if nblk % s == 0:
                    slab = s
                    break
            x_r = x_in.rearrange("(nb p) f -> p nb f", p=P)
            t1_r = t1_shard.rearrange("(nb p) f -> p nb f", p=P)
            for s0 in range(0, nblk, slab):
                xb = xpool.tile([P, slab, P], f32, tag="xb")
                nc.sync.dma_start(out=xb[:], in_=x_r[:, s0:s0 + slab, :])
                t1b = xpool.tile([P, slab, P], bf16, tag="t1b")
                for j in range(slab):
                    nc.vector.tensor_tensor(
                        out=t1b[:, j, :],
                        in0=xb[:, j, :],
                        in1=dinv[:, s0 + j:s0 + j + 1].to_broadcast([P, P]),
                        op=mybir.AluOpType.mult)
                nc.scalar.dma_start(out=t1_r[:, s0:s0 + slab, :], in_=t1b[:])

            nc.gpsimd.collective_compute(
                "AllGather", mybir.AluOpType.bypass,
                replica_groups=[list(range(NCORE))],
                ins=[t1_shard[:]], outs=[t1_full[:]])

            pool_lo = ppool.tile([P, P], f32, space="PSUM")
            pool_hi = ppool.tile([P, P], f32, space="PSUM")

            def sweep(layer, lx, idx_sb, rl_sb, t_full_d, t_shard_d, w_sb):
                calls = lx["calls"]
                # layer 1 fires the t2 AllGather per quarter as soon as the
                # producing blocks are done, hiding it under the sweep.
                fire_after = {}
                if layer == 1:
                    for q in range(cfg.nag):
                        bb = ((q + 1) * cfg.hrows - 1) // P
                        fire_after.setdefault(bb, []).append(q)

                def maybe_fire_ag(b):
                    for q in fire_after.get(b, ()):
                        nc.gpsimd.collective_compute(
                            "AllGather", mybir.AluOpType.bypass,
                            replica_groups=[list(range(NCORE))],
                            ins=[t2_shard[q * cfg.hrows:(q + 1) * cfg.hrows, :]],
                            outs=[t2_full[q * NCORE * cfg.hrows:
                                          (q + 1) * NCORE * cfg.hrows, :]])

                agg = {}
                call_i = 0
                for sb in range(nsb):
                    blocks = list(range(sb * cfg.sb_blocks,
                                        min((sb + 1) * cfg.sb_blocks, nblk)))
                    # self-loop contribution opens each block's accumulation
                    for b in blocks:
                        tloc = bpool.tile([P, P], bf16, tag="tloc")
                        nc.sync.dma_start(
                            out=tloc[:], in_=t_shard_d[b * P:(b + 1) * P, :])
                        ap = apool.tile([P, P], f32, tag="agg", space="PSUM")
                        agg[b] = ap
                        nc.tensor.matmul(ap[:], lhsT=tloc[:], rhs=ident[:],
                                         start=True, stop=False,
                                         skip_group_check=True)
                    while call_i < len(calls) and calls[call_i]["sb"] == sb:
                        call = calls[call_i]
                        ncol, cap, t = call["ncol"], call["cap"], call["t"]
                        g_sb = gpool.tile([P, ncol, P], bf16, tag="g")
                        nc.gpsimd.dma_gather(
                            g_sb[:],
                            t_full_d[t * cfg.tab_rows:(t + 1) * cfg.tab_rows, :],
                            idx_sb[:, call["icol"]:call["icol"] + cap // 16],
                            cap, cap, P,
                            single_packet=False, queue_num=t % 4)
                        for b, coli, p0, p1, pci, is_last in call["pieces"]:
                            s_sb = spool.tile([P, P], bf16, tag="s")
                            nc.vector.tensor_tensor(
                                out=s_sb[:],
                                in0=iota_lo[:],
                                in1=rl_sb[:, pci:pci + 1].to_broadcast([P, P]),
                                op=mybir.AluOpType.is_equal)
                            nc.tensor.matmul(
                                agg[b][:], lhsT=g_sb[:, coli, :],
                                rhs=s_sb[:],
                                start=False, stop=is_last,
                                skip_group_check=True)
                        call_i += 1
                    # finalize blocks of this superblock
                    for b in blocks:
                        aggT = bpool.tile([P, P], bf16, tag="aggT")
                        nc.scalar.copy(out=aggT[:], in_=agg[b][:])
                        hp = hpool.tile([P, P], f32, tag="h", space="PSUM")
                        nc.tensor.matmul(hp[:], lhsT=aggT[:], rhs=w_sb[:],
                                         start=True, stop=True,
                                         skip_group_check=True)
                        if layer == 1:
                            t2b = bpool.tile([P, P], bf16, tag="t2b")
                            nc.scalar.activation(
                                out=t2b[:], in_=hp[:],
                                func=mybir.ActivationFunctionType.Relu,
                                scale=dinv2[:, b:b + 1])
                            nc.sync.dma_start(
                                out=t2_shard[b * P:(b + 1) * P, :], in_=t2b[:])
                            maybe_fire_ag(b)
                        else:
                            o2 = bpool.tile([P, P], bf16, tag="o2")
                            nc.scalar.activation(
                                out=o2[:], in_=hp[:],
                                func=mybir.ActivationFunctionType.Copy,
                                scale=dinv[:, b:b + 1])
                            plo = spool.tile([P, P], bf16, tag="plo")
                            nc.vector.tensor_tensor(
                                out=plo[:], in0=iota_lo[:],
                                in1=batch_sb[:, b:b + 1].to_broadcast([P, P]),
                                op=mybir.AluOpType.is_equal)
                            nc.tensor.matmul(pool_lo[:], lhsT=plo[:], rhs=o2[:],
                                             start=(b == 0), stop=(b == nblk - 1),
                                             skip_group_check=True)
                            phi = spool.tile([P, P], bf16, tag="phi")
                            nc.vector.tensor_tensor(
                                out=phi[:], in0=iota_hi[:],
                                in1=batch_sb[:, b:b + 1].to_broadcast([P, P]),
                                op=mybir.AluOpType.is_equal)
                            nc.tensor.matmul(pool_hi[:], lhsT=phi[:], rhs=o2[:],
                                             start=(b == 0), stop=(b == nblk - 1),
                                             skip_group_check=True)

            sweep(1, l1, idx1_sb, rl1_sb, t1_full, t1_shard, w1_sb)
            sweep(2, l2, idx2_sb, rl2_sb, t2_full, t2_shard, w2_sb)

            # ---- pool partials -> AllReduce -> divide ----
            for j, pt in enumerate((pool_lo, pool_hi)):
                ps = xpool.tile([P, P], f32, tag="ps")
                nc.vector.tensor_copy(out=ps[:], in_=pt[:])
                nc.sync.dma_start(out=pool_part[j * P:(j + 1) * P, :], in_=ps[:])
            nc.gpsimd.collective_compute(
                "AllReduce", mybir.AluOpType.add,
                replica_groups=[list(range(NCORE))],
                ins=[pool_part[:]], outs=[pool_full[:]])
            for j in range(2):
                pf = xpool.tile([P, P], f32, tag="pf")
                nc.sync.dma_start(out=pf[:], in_=pool_full[j * P:(j + 1) * P, :])
                of = xpool.tile([P, P], f32, tag="of")
                nc.vector.tensor_tensor(
                    out=of[:], in0=pf[:],
                    in1=invc_sb[:, j:j + 1].to_broadcast([P, P]),
                    op=mybir.AluOpType.mult)
                nc.sync.dma_start(out=out_ext[j * P:(j + 1) * P, :], in_=of[:])

    nc.compile()
    return nc


def make_in_maps(cfg, prep, x, W1, W2):
    x = np.asarray(x, dtype=np.float32)
    iota = np.broadcast_to(np.arange(P, dtype=np.float32), (P, P))
    iota_hi = iota + P
    ident = np.eye(P, dtype=np.float32)
    in_maps = []
    for c in range(NCORE):
        pc = prep["perm"][c]
        valid = pc >= 0
        xl = np.zeros((cfg.n_shp, P), dtype=np.float32)
        xl[valid] = x[c * cfg.n_sh + pc[valid]]
        in_maps.append({
            "x_local": xl,
            "w1": np.asarray(W1, dtype=np.float32),
            "w2": np.asarray(W2, dtype=np.float32),
            "deg_t": prep["deg_t"][c],
            "iota_lo": np.ascontiguousarray(iota).astype(ml_dtypes.bfloat16),
            "iota_hi": np.ascontiguousarray(iota_hi).astype(ml_dtypes.bfloat16),
            "ident": ident.astype(ml_dtypes.bfloat16),
            "idx1": prep["l1"]["idx_tiles"][c],
            "rl1": prep["l1"]["rl_tiles"][c],
            "idx2": prep["l2"]["idx_tiles"][c],
            "rl2": prep["l2"]["rl_tiles"][c],
            "batch_t": prep["batch_t"][c],
            "inv_cnt": prep["inv_tile"],
        })
    return in_maps


def run(x, edge_index, batch, num_graphs, W1, b1, W2, b2, trace=False):
    from concourse.bass_utils import run_bass_kernel_spmd
    N = int(x.shape[0])
    G = int(num_graphs)
    assert not np.any(np.asarray(b1)) and not np.any(np.asarray(b2)), \
        "nonzero bias not supported"
    cfg = Cfg(N, G)
    prep = host_prep(cfg, np.asarray(edge_index), np.asarray(batch))
    nc = build_program(cfg, prep)
    in_maps = make_in_maps(cfg, prep, x, W1, W2)
    res = run_bass_kernel_spmd(nc, in_maps, list(range(NCORE)), trace=trace)
    out = res.results[0]["out"][:G].astype(np.float32)
    return out, res


def kernel(x, edge_index, batch, num_graphs, W1, b1, W2, b2):
    """Full-input entry point: takes the unsharded problem, distributes it
    across 8 NeuronCores internally, returns the pooled [num_graphs, 128]
    float32 output."""
    out, _ = run(np.asarray(x), np.asarray(edge_index), np.asarray(batch),
                 int(num_graphs), np.asarray(W1), b1, np.asarray(W2), b2)
    return out

